# revision 24
# baseline (speedup 1.0000x reference)
"""Trainium2 Bass kernel for nn_MinimalAdderNN.

Computation (see reference): a 10-digit ripple-carry adder over base-10 digit
tensors a, b of shape [1048576, 10] (int32, digits 0..9), using two lookup
tables built deterministically by setup_inputs(). For those structured tables
the output rows are exact one-hots:
    out[n, 1+p, k] = (k == (a[n,p] + b[n,p] + carry_in) % 10)
    out[n, 0,   k] = (k == final_carry)
so the kernel computes digits/carries arithmetically on the VectorEngine:
    s = a + b  ->  carry chain via one tensor_tensor_scan (pad slot resets
    state between batch elements)  ->  d = (s + c) mod 10  ->  one-hot via
    is_equal against an iota, written straight to the output layout.

Sharding: pure data-parallel over batch across 8 NeuronCores (131072 rows
per core); the tables are consumed host-side only (validated against the
expected structured tables).
"""
import sys

sys.path.insert(0, "/opt/trn_rl_repo")

import numpy as np

import concourse.bass as bass
import concourse.bacc as bacc
import concourse.mybir as mybir
import concourse.tile as tile
from concourse.bass_utils import run_bass_kernel_spmd

BATCH = 1048576
D = 10
NCORES = 8
NPC = BATCH // NCORES  # 131072 rows per core
P = 128
# Variable tile sizes (batch elems per partition per tile): small head/tail
# tiles shorten pipeline fill/drain; must sum to NPC // P = 1024.
QS = [64] * 16
PW = D + 1             # padded slots per batch elem
OW = (D + 1) * D       # 110 output floats per batch elem

f32 = mybir.dt.float32
i32 = mybir.dt.int32
i8 = mybir.dt.int8
bf16 = mybir.dt.bfloat16

_CACHE = {}


def _expected_tables():
    next_carry = np.zeros((200, 2), dtype=np.float32)
    digit = np.zeros((200, 10), dtype=np.float32)
    for carry in (0, 1):
        for a_ in range(10):
            for b_ in range(10):
                idx = carry * 100 + a_ * 10 + b_
                total = a_ + b_ + carry
                next_carry[idx, total // 10] = 1.0
                digit[idx, total % 10] = 1.0
    return digit, next_carry


def _tables_are_structured(digit_table, carry_table):
    digit_exp, carry_exp = _expected_tables()
    if digit_table.shape != (200, 10) or carry_table.shape != (200, 2):
        return False
    if not np.array_equal(digit_table, digit_exp):
        return False
    # The reference only consumes argmax(carry_table[idx]); the fast path is
    # valid iff that argmax equals the arithmetic carry bit for every index.
    bits = np.argmax(carry_table, axis=1)
    return np.array_equal(bits, np.argmax(carry_exp, axis=1))


def _build_fast_nc():
    assert sum(QS) * P == NPC
    qmax = max(QS)
    nc = bacc.Bacc()
    ab_d = nc.dram_tensor("ab", [2, NPC, D], i8, kind="ExternalInput").ap()
    o_d = nc.dram_tensor("out", [NPC, OW], f32, kind="ExternalOutput").ap()

    with tile.TileContext(nc) as tc:
        with tc.tile_pool(name="const", bufs=1) as cp, \
             tc.tile_pool(name="io", bufs=8) as iop, \
             tc.tile_pool(name="wk", bufs=2) as wp, \
             tc.tile_pool(name="ot", bufs=4) as op_:
            iota = cp.tile([P, OW], f32, tag="iota")
            nc.gpsimd.iota(iota[:], pattern=[[0, PW], [1, D]], base=0,
                           channel_multiplier=0,
                           allow_small_or_imprecise_dtypes=True)
            tens = cp.tile([P, qmax * PW], f32, tag="tens")
            nc.vector.memset(tens[:], 10.0)
            bias_t = cp.tile([P, D], f32, tag="bias")
            for k in range(D):
                nc.vector.memset(bias_t[:, k:k + 1], -float(k))

            r0 = 0
            for t, Q in enumerate(QS):
                ab_src = ab_d[:, r0:r0 + P * Q, :] \
                    .rearrange("z (p q) d -> p z (q d)", q=Q)
                o_dst = o_d[r0:r0 + P * Q, :] \
                    .rearrange("(p q) d -> p (q d)", q=Q)
                r0 += P * Q

                abt = iop.tile([P, 2 * qmax * D], i8, tag="ab")
                nc.scalar.dma_start(
                    abt[:, :2 * Q * D].rearrange("p (z f) -> p z f", z=2),
                    ab_src)

                s_pad = wp.tile([P, qmax * PW], f32, tag="s")
                sp3 = s_pad[:, :Q * PW].rearrange("p (q e) -> p q e", e=PW)
                nc.gpsimd.memset(sp3[:, :, 0:1], 0.0)
                ab4 = abt[:, :2 * Q * D].rearrange("p (z q d) -> p z q d",
                                                   z=2, d=D)
                a3 = ab4[:, 0]
                b3 = ab4[:, 1]
                # s_pad[q, 1+j] = a[q, 9-j] + b[q, 9-j]  (chain: LSD first)
                nc.vector.tensor_tensor(sp3[:, :, 1:PW], a3[:, :, ::-1],
                                        b3[:, :, ::-1], op=mybir.AluOpType.add)

                c = wp.tile([P, qmax * PW], f32, tag="c")
                nc.vector.tensor_tensor_scan(c[:, :Q * PW], s_pad[:, :Q * PW],
                                             tens[:, :Q * PW], 0.0,
                                             op0=mybir.AluOpType.add,
                                             op1=mybir.AluOpType.is_ge)
                c3 = c[:, :Q * PW].rearrange("p (q e) -> p q e", e=PW)

                tt = wp.tile([P, qmax * D], f32, tag="t")
                t3 = tt[:, :Q * D].rearrange("p (q d) -> p q d", d=D)
                nc.vector.tensor_tensor(t3[:, :, :], sp3[:, :, 1:PW],
                                        c3[:, :, 0:D], op=mybir.AluOpType.add)
                # d = t - 10*c_out written in-place over the consumed carry-in
                # slots of c (c then holds [d0..d9, final_carry] = VAL).
                nc.vector.scalar_tensor_tensor(
                    c3[:, :, 0:D], c3[:, :, 1:PW], -10.0, t3[:, :, :],
                    op0=mybir.AluOpType.mult, op1=mybir.AluOpType.add)
                v3 = c3

                ot = op_.tile([P, qmax * OW], f32, tag="o")
                o4 = ot[:, :Q * OW].rearrange("p (q w k) -> p q w k",
                                              w=PW, k=D)
                v_rev = v3[:, :, ::-1]
                v_bc = v_rev.unsqueeze(3).broadcast_to([P, Q, PW, D])
                i4 = iota[:].rearrange("p (w k) -> p w k", k=D) \
                    .unsqueeze(1).broadcast_to([P, Q, PW, D])
                # one-hot: ScalarE takes the first SA positions via exact
                # relu(1 - (x-k)^2); VectorE takes the rest via is_equal.
                SA = 4
                for k in range(D):
                    nc.scalar.activation(
                        o4[:, :, 0:SA, k:k + 1].squeeze(3), v_rev[:, :, 0:SA],
                        mybir.ActivationFunctionType.Square,
                        bias=bias_t[:, k:k + 1], scale=1.0)
                oa = o4[:, :, 0:SA, :]
                nc.scalar.activation(
                    oa, oa, mybir.ActivationFunctionType.Relu,
                    bias=1.0, scale=-1.0)
                nc.vector.tensor_tensor(o4[:, :, SA:PW, :], v_bc[:, :, SA:PW],
                                        i4[:, :, SA:PW],
                                        op=mybir.AluOpType.is_equal)

                nc.sync.dma_start(o_dst, ot[:, :Q * OW])
    nc.compile()
    return nc


def _run_fast(a, b, trace=False, trace_kwargs=None):
    if "fast_nc" not in _CACHE:
        _CACHE["fast_nc"] = _build_fast_nc()
    nc = _CACHE["fast_nc"]
    in_maps = []
    for cid in range(NCORES):
        sl = slice(cid * NPC, (cid + 1) * NPC)
        in_maps.append({"ab": np.ascontiguousarray(
            np.stack([a[sl], b[sl]], axis=0)).astype(np.int8)})
    res = run_bass_kernel_spmd(nc, in_maps, core_ids=list(range(NCORES)),
                               trace=trace, **(trace_kwargs or {}))
    out = np.concatenate([r["out"] for r in res.results], axis=0)
    return out.reshape(BATCH, D + 1, D), res


def _run_general_host(a, b, digit_table, carry_table):
    # Correctness fallback for non-structured tables (not expected from the
    # reference's setup_inputs); computed host-side.
    n = a.shape[0]
    carry = np.zeros(n, dtype=np.int64)
    out = np.empty((n, D + 1, D), dtype=digit_table.dtype)
    for p in range(D - 1, -1, -1):
        idx = carry * 100 + a[:, p].astype(np.int64) * 10 + b[:, p].astype(np.int64)
        out[:, 1 + p, :] = digit_table[idx]
        carry = np.argmax(carry_table[idx], axis=1)
    lead = np.zeros((n, D), dtype=digit_table.dtype)
    lead[np.arange(n), carry] = 1.0
    out[:, 0, :] = lead
    return out


def kernel(a, b, digit_table, carry_table):
    a = np.asarray(a, dtype=np.int32)
    b = np.asarray(b, dtype=np.int32)
    digit_table = np.asarray(digit_table, dtype=np.float32)
    carry_table = np.asarray(carry_table, dtype=np.float32)
    assert a.shape == (BATCH, D) and b.shape == (BATCH, D), (a.shape, b.shape)
    if _tables_are_structured(digit_table, carry_table):
        out, _ = _run_fast(a, b)
        return out
    return _run_general_host(a, b, digit_table, carry_table)
